# revision 9
# baseline (speedup 1.0000x reference)
"""CTC loss (keras ctc_batch_cost semantics) on 8 Trainium2 NeuronCores.

Strategy (pure data parallelism, batch sharded 128 samples/core):
  - DP runs in probability space with periodic per-sample rescaling:
        P[t,s] = y_ext[t,s] * (P[t-1,s] + P[t-1,s-1] + allow_skip*P[t-1,s-2])
    Samples ride the 128 SBUF partitions; the S=129 lattice states live in
    the free dimension of [128, S]-shaped DVE ops.
  - y_pred is transposed on the HOST to [chunk, c_half, c, b, t] so each
    (chunk, half) is a single 4MB DMA with 32KB contiguous partition
    lines (no DMA transposes anywhere).
  - The per-(sample,t) emission gather y_pred[b,t,ext(b,s)] is done with
    per-sample one-hot matmuls on the PE array:
        G[b] = W[b].T @ yT[b]   with W[b] [C,128] = packed one-hots:
            cols 0..63  : onehot(lab[l]) * allow_skip * e^-2g  (skip)
            cols 64..127: onehot(lab[l])                       (label)
    EPS is folded into the PSUM->SBUF staging copy (ACT bias).
    Per time step a PE transpose turns G[:, (q si), t] into a
    [128b, 128m] tile the DVE consumes directly from PSUM.
  - Blank emissions (even lattice states) multiply by a per-partition scalar
    plane ybe[b,t] = y_pred[b,t,C-1]+EPS (host-extracted).
  - Loss = -(log(P[2L] + P[2L-1]) + sum of rescale logs).
"""

import numpy as np

B, T, C, L = 1024, 512, 256, 64
S = 2 * L + 1  # 129
NCORES = 8
BL = B // NCORES  # 128 samples per core
EPS = 1e-7
RBLK = 8  # rescale period (time steps)
# Static per-state exponential tilt P~[s] = P[s]*exp(-G_TILT*s). Flattens the
# lattice's s-profile so all answer-relevant states fit f32 range; folded into
# the sh1 scalar, the host-built W/end-mask, and the logacc initialization.
G_TILT = 1.75
OFFS = 30.0  # rescale offset: row max is normalized to e^OFFS, not 1

_prog = None  # cached compiled Bass program
_last_results = None


def _build_program():
    from contextlib import ExitStack

    import concourse.bacc as bacc
    import concourse.bass as bass
    import concourse.mybir as mybir
    import concourse.tile as tile

    F32 = mybir.dt.float32
    BF16 = mybir.dt.bfloat16
    OP = mybir.AluOpType
    AF = mybir.ActivationFunctionType
    AX = mybir.AxisListType
    PSUM = bass.MemorySpace.PSUM

    TCH = 128            # time-chunk length
    NCH = T // TCH       # 4 chunks
    NQ = BL // 4         # 32 sample quads per chunk
    NG = NQ // 4         # 8 quad-groups (wg DMA granularity)
    E1 = float(np.exp(-G_TILT))
    OFFE = float(np.exp(OFFS))

    nc = bacc.Bacc("TRN2", target_bir_lowering=False, debug=False)

    # ypt[k,h,c,b,t] = y_pred[b, k*TCH+t, h*128+c] (bf16, host-transposed)
    ypt_d = nc.dram_tensor("ypt", [NCH, 2, 128, BL, TCH], BF16,
                           kind="ExternalInput").ap()
    # wgg[g,c,(q4 si m)]: 4 quads' packed one-hot weights per group
    wgg_d = nc.dram_tensor("wgg", [NG, 128, 4 * 4 * 256], BF16,
                           kind="ExternalInput").ap()
    ybe_d = nc.dram_tensor("ybe", [BL, T], F32, kind="ExternalInput").ap()
    em_d = nc.dram_tensor("em", [BL, S], F32, kind="ExternalInput").ap()
    idf_d = nc.dram_tensor("idf", [128, 128], BF16, kind="ExternalInput").ap()
    pend_d = nc.dram_tensor("pend", [BL, 1], F32, kind="ExternalOutput").ap()
    mxh_d = nc.dram_tensor("mxh", [BL, T // RBLK], F32, kind="ExternalOutput").ap()

    with tile.TileContext(nc) as tc, ExitStack() as ctx:
        # ---- persistent SBUF state (one pool, unique tags) ----
        per = ctx.enter_context(tc.tile_pool(name="per", bufs=1))
        ybe_sb = per.tile([128, T], F32, tag="ybe", name="ybe_sb")
        em_sb = per.tile([128, S], F32, tag="em", name="em_sb")
        idf = per.tile([128, 128], BF16, tag="idf", name="idf_sb")
        pa = per.tile([128, 264], F32, tag="pa", name="pa")
        pb = per.tile([128, 264], F32, tag="pb", name="pb")
        mxh = per.tile([128, T // RBLK], F32, tag="mxh", name="mxh")

        nc.sync.dma_start(ybe_sb[:], ybe_d)
        nc.sync.dma_start(em_sb[:], em_d)
        nc.sync.dma_start(idf[:], idf_d)
        nc.vector.memset(pa[:], 0.0)
        nc.vector.memset(pb[:], 0.0)

        # ---- pools ----
        ypl = ctx.enter_context(tc.tile_pool(name="ypl", bufs=3))
        wpl = ctx.enter_context(tc.tile_pool(name="wpl", bufs=3))
        gcp = ctx.enter_context(tc.tile_pool(name="gcp", bufs=2))
        vpl = ctx.enter_context(tc.tile_pool(name="vpl", bufs=3))
        spl = ctx.enter_context(tc.tile_pool(name="spl", bufs=6))
        gpp = ctx.enter_context(tc.tile_pool(name="gpp", space=PSUM, bufs=3))
        yyp = ctx.enter_context(tc.tile_pool(name="yyp", space=PSUM, bufs=5))

        gc_t = {}   # chunk -> gather SBUF tile [128m, (q si t)] (bf16)
        yp_t = {}   # chunk -> (lo, hi) staged y tiles [128c, (b t)] (bf16)
        wg_t = {}   # (chunk, group) -> weight tile [128c, (q4 si m)]

        def ypt_load(k):
            lo = ypl.tile([128, BL * TCH], BF16, tag="ypt", name="yplo")
            hi = ypl.tile([128, BL * TCH], BF16, tag="ypt", name="yphi")
            nc.sync.dma_start(lo[:], ypt_d[k, 0].rearrange("c b t -> c (b t)"))
            nc.sync.dma_start(hi[:], ypt_d[k, 1].rearrange("c b t -> c (b t)"))
            yp_t[k] = (lo, hi)

        def gather_open(k):
            g = gcp.tile([128, TCH * 128], BF16, tag="gc", name="gc")
            gc_t[k] = g

        def wg_load(k, grp):
            w = wpl.tile([128, 4 * 4 * 256], BF16, tag="w", name="w")
            nc.sync.dma_start(w[:], wgg_d[grp])
            wg_t[(k, grp)] = w

        gq_t = {}  # chunk -> current quad's PSUM tile

        def gather_si(k, q, si):
            # one sample's pair of matmuls; quad's PSUM tile opens at si=0
            # and is staged to SBUF (with +EPS via ACT bias) at si=3
            if si == 0:
                gq_t[k] = gpp.tile([128, 512], F32, tag="gq", name="gq")
            gq = gq_t[k]
            w4 = wg_t[(k, q // 4)][:].rearrange(
                "c (q4 si m) -> c q4 si m", q4=4, si=4)[:, q % 4]
            lo, hi = yp_t[k]
            smp = q * 4 + si
            sl = slice(si * 128, (si + 1) * 128)
            ysl = slice(smp * TCH, (smp + 1) * TCH)
            nc.tensor.matmul(gq[:, sl], w4[:, si, 0:128], lo[:, ysl],
                             start=True, stop=False)
            nc.tensor.matmul(gq[:, sl], w4[:, si, 128:256], hi[:, ysl],
                             start=False, stop=True)
            if si == 3:
                nc.scalar.activation(gc_t[k][:, q * 512:(q + 1) * 512], gq[:],
                                     AF.Copy, bias=EPS)

        def gather_quad(k, q):
            for si in range(4):
                gather_si(k, q, si)

        def yy_view(k, tl):
            # [128m, 128b] view of gc_t[k] at time tl: b = q*4+si,
            # element (q,si) at col q*512 + si*128 + tl
            g = gc_t[k][:]
            return bass.AP(g.tensor, g.offset + tl,
                           [g.ap[0], [512, 32], [128, 4]])

        AOFF = 134  # A[s] lives at col AOFF+s of the *current* state tensor

        def dp_step(t, pcur, pnxt, rec2, csc):
            k, tl = divmod(t, TCH)
            yy = yyp.tile([128, 128], BF16, tag="yy")
            nc.tensor.transpose(yy[:], yy_view(k, tl), idf[:])
            # A[s] = P[s] + e^-g*P[s-1], written into pcur's scratch region
            nc.vector.scalar_tensor_tensor(pcur[:, AOFF:AOFF + 129],
                                           pcur[:, 0:129], E1,
                                           pcur[:, 1:130], OP.mult, OP.add)
            u3 = pnxt[:, 1:131].rearrange("p (s two) -> p s two", two=2)
            a_even = pcur[:, AOFF:AOFF + 130].rearrange(
                "p (s two) -> p s two", two=2)[:, :, 0]
            # even states on ACT: A_even * ybe (csc = ybe*rec2 post-rescale)
            nc.scalar.activation(u3[:, :, 0], a_even, AF.Copy, bias=0.0,
                                 scale=(csc if csc is not None
                                        else ybe_sb[:, t:t + 1]))
            # one 2D-strided multiply covers skip & label terms:
            #   X[0,l] = P[2l]     * yy[0..63]   (skip: e^-2g * masked onehot)
            #   X[1,l] = A[2l+1]   * yy[64..127] (label emission)
            stz = bass.AP(pcur[:].tensor, pcur[:].offset,
                          [pcur[:].ap[0], [AOFF + 1, 2], [2, 64]])
            x = vpl.tile([128, 128], F32, tag="x")
            if rec2 is None:
                nc.vector.tensor_tensor(x[:], stz, yy[:], OP.mult)
            else:
                nc.vector.scalar_tensor_tensor(x[:], stz, rec2[:], yy[:],
                                               OP.mult, OP.mult)
            nc.vector.tensor_tensor(u3[:, 0:64, 1], x[:, 0:64], x[:, 64:128],
                                    OP.add)
            if t % RBLK == RBLK - 1:
                ridx = t // RBLK
                mxc = mxh[:, ridx:ridx + 1]
                nc.vector.tensor_reduce(mxc, pnxt[:, 1:130], AX.X, OP.max)
                rec = spl.tile([128, 1], F32, tag="rec")
                nc.vector.reciprocal(rec[:], mxc)
                rec2n = spl.tile([128, 1], F32, tag="rec2")
                nc.vector.tensor_scalar(rec2n[:], rec[:], OFFE, None, OP.mult)
                if t + 1 >= T:
                    return rec2n, None
                # combined even-state scale for step t+1, built on ACT
                cscn = spl.tile([128, 1], F32, tag="csc")
                nc.scalar.activation(cscn[:], rec2n[:], AF.Copy, bias=0.0,
                                     scale=ybe_sb[:, t + 1:t + 2])
                return rec2n, cscn
            return None, None

        # chunk 0: load + gather before the DP loop starts
        ypt_load(0)
        gather_open(0)
        for grp in range(NG):
            wg_load(0, grp)
        for q in range(NQ):
            gather_quad(0, q)

        # init (t = 0): P[s=0] = ybe[:,0]; P~[s=1] = e^-g * y_lab(l=0,t=0)
        yy0 = yyp.tile([128, 128], BF16, tag="yy")
        nc.tensor.transpose(yy0[:], yy_view(0, 0), idf[:])
        nc.vector.tensor_copy(pa[:, 1:2], ybe_sb[:, 0:1])
        nc.vector.tensor_scalar(pa[:, 2:3], yy0[:, 64:65], E1, None, OP.mult)

        pcur, pnxt = pa, pb
        rec2, csc = None, None
        for t in range(1, T):
            k, tl = divmod(t, TCH)
            # interleave next-chunk gather emission through this chunk's DP
            # steps (one sample's 2 matmuls per step) so the PE never falls
            # far behind on the per-step yy transposes
            if k + 1 < NCH:
                if k == 0:
                    # chunk 0 has 127 DP steps; pack 2 samples per step
                    if tl == 1:
                        ypt_load(1)
                        gather_open(1)
                    if tl % 8 == 1 and tl <= 57:
                        wg_load(1, tl // 8)
                    if tl <= 64:
                        u = 2 * (tl - 1)
                        gather_si(1, u // 4, u % 4)
                        gather_si(1, (u + 1) // 4, (u + 1) % 4)
                else:
                    if tl == 0:
                        ypt_load(k + 1)
                        gather_open(k + 1)
                    if tl % 16 == 0:
                        wg_load(k + 1, tl // 16)
                    gather_si(k + 1, tl // 4, tl % 4)
            rec2, csc = dp_step(t, pcur, pnxt, rec2, csc)
            pcur, pnxt = pnxt, pcur
        if rec2 is not None:
            # the last rescale's scaling never got absorbed; apply it now
            nc.vector.tensor_scalar_mul(pcur[:, 1:130], pcur[:, 1:130], rec2[:])

        # final: export pend = sum(P * endmask) and the rescale history;
        # the exact logs happen on the host.
        scre = per.tile([128, S], F32, tag="scre", name="scre")
        nc.vector.tensor_tensor(scre[:], pcur[:, 1:130], em_sb[:], OP.mult)
        pend = per.tile([128, 1], F32, tag="pend", name="pend")
        nc.vector.tensor_reduce(pend[:], scre[:], AX.X, OP.add)
        nc.sync.dma_start(pend_d, pend[:])
        nc.sync.dma_start(mxh_d, mxh[:])

    nc.compile()
    return nc


def _host_derived(y_true, y_pred, label_length):
    import ml_dtypes

    lab = np.asarray(y_true, dtype=np.int64)  # [B, 64]
    llv = np.asarray(label_length).reshape(-1)
    # packed one-hots: [B, C, 128]; cols 0..63 skip-masked labels scaled by
    # e^(-2g), cols 64..127 labels (validity-masked)
    vm = (np.arange(L)[None, :] < llv[:, None])  # valid odd state s=2l+1
    zm = np.concatenate([np.zeros((B, 1), bool), lab[:, 1:] != lab[:, :-1]], axis=1)
    w = np.zeros((B, C, 128), dtype=np.float32)
    bb = np.repeat(np.arange(B), L)
    ll = np.tile(np.arange(L), B)
    cc = lab.reshape(-1)
    w[bb, cc, L + ll] = vm.reshape(-1).astype(np.float32)
    w[bb, cc, ll] = np.where(
        (zm & vm).reshape(-1),
        np.float32(np.exp(-2.0 * G_TILT)),
        w[bb, cc, ll],
    )
    # device layout: [group, 128c(lo), 4q, 4si, (ck m)] with c = ck*128 + c_lo
    w6 = w.reshape(B // 16, 4, 4, 2, 128, 128)      # [g, q4, si, ck, c_lo, m]
    w6 = w6.transpose(0, 4, 1, 2, 3, 5)             # [g, c_lo, q4, si, ck, m]
    wgg = np.ascontiguousarray(
        w6.reshape(B // 16, 128, 4 * 4 * 256).astype(ml_dtypes.bfloat16)
    )
    ybe = np.ascontiguousarray(np.asarray(y_pred)[:, :, C - 1] + np.float32(EPS))
    return wgg, ybe


def kernel(y_true, y_pred, input_length, label_length, _trace=False):
    global _prog, _last_results
    from concourse.bass_utils import run_bass_kernel_spmd

    y_true = np.asarray(y_true)
    import ml_dtypes
    y_pred = np.asarray(y_pred, dtype=np.float32)
    y_pred_bf = y_pred.astype(ml_dtypes.bfloat16)
    label_length = np.asarray(label_length).reshape(-1)

    TCH = 128
    NCH = T // TCH
    # ypt[core][k,h,c,b,t] = y_pred[b, k*TCH+t, h*128+c]
    ypt = y_pred_bf.reshape(NCORES, BL, NCH, TCH, 2, 128).transpose(
        0, 2, 4, 5, 1, 3)
    ypt = np.ascontiguousarray(ypt)

    wgg, ybe = _host_derived(y_true, y_pred, label_length)
    em = np.zeros((B, S), dtype=np.float32)
    bidx = np.arange(B)
    em[bidx, 2 * label_length] = 1.0
    em[bidx, 2 * label_length - 1] = np.float32(np.exp(-G_TILT))
    idf = np.eye(128, dtype=ml_dtypes.bfloat16)

    if _prog is None:
        _prog = _build_program()

    NGC = BL // 16  # wg groups per core
    in_maps = []
    for i in range(NCORES):
        sl = slice(i * BL, (i + 1) * BL)
        slg = slice(i * NGC, (i + 1) * NGC)
        in_maps.append({
            "ypt": ypt[i],
            "wgg": wgg[slg],
            "ybe": ybe[sl],
            "em": em[sl],
            "idf": idf,
        })
    res = run_bass_kernel_spmd(_prog, in_maps, core_ids=list(range(NCORES)),
                               trace=_trace)
    _last_results = res
    pend = np.concatenate([r["pend"] for r in res.results], axis=0).reshape(-1)
    mxh = np.concatenate([r["mxh"] for r in res.results], axis=0)
    nres = mxh.shape[1]
    logacc = np.log(mxh.astype(np.float64)).sum(axis=1) - OFFS * nres
    loss = -(np.log(pend.astype(np.float64)) + logacc
             + G_TILT * 2.0 * label_length.astype(np.float64))
    return loss.reshape(B, 1).astype(np.float32)


if __name__ == "__main__":
    rng = np.random.default_rng(0)
    yp = rng.random((B, T, C), dtype=np.float32)
    yp /= yp.sum(-1, keepdims=True)
    yt = rng.integers(0, C - 1, size=(B, L)).astype(np.int32)
    il = np.full((B, 1), T, dtype=np.int32)
    ll = rng.integers(32, L + 1, size=(B, 1)).astype(np.int32)
    print(kernel(yt, yp, il, ll)[:4])


# revision 12
# speedup vs baseline: 1.0409x; 1.0409x over previous
"""CTC loss (keras ctc_batch_cost semantics) on 8 Trainium2 NeuronCores.

Strategy (pure data parallelism, batch sharded 128 samples/core):
  - DP runs in probability space with periodic per-sample rescaling:
        P[t,s] = y_ext[t,s] * (P[t-1,s] + P[t-1,s-1] + allow_skip*P[t-1,s-2])
    Samples ride the 128 SBUF partitions; the S=129 lattice states live in
    the free dimension of [128, S]-shaped DVE ops.
  - y_pred is transposed on the HOST to [chunk, c_half, c, b, t] so each
    (chunk, half) is a single 4MB DMA with 32KB contiguous partition
    lines (no DMA transposes anywhere).
  - The per-(sample,t) emission gather y_pred[b,t,ext(b,s)] is done with
    per-sample one-hot matmuls on the PE array:
        G[b] = W[b].T @ yT[b]   with W[b] [C,128] = packed one-hots:
            cols 0..63  : onehot(lab[l]) * allow_skip * e^-2g  (skip)
            cols 64..127: onehot(lab[l])                       (label)
    EPS is folded into the PSUM->SBUF staging copy (ACT bias).
    Per time step a PE transpose turns G[:, (q si), t] into a
    [128b, 128m] tile the DVE consumes directly from PSUM.
  - Blank emissions (even lattice states) multiply by a per-partition scalar
    plane ybe[b,t] = y_pred[b,t,C-1]+EPS (host-extracted).
  - Loss = -(log(P[2L] + P[2L-1]) + sum of rescale logs).
"""

import numpy as np

B, T, C, L = 1024, 512, 256, 64
S = 2 * L + 1  # 129
NCORES = 8
BL = B // NCORES  # 128 samples per core
EPS = 1e-7
RBLK = 8  # rescale period (time steps)
# Static per-state exponential tilt P~[s] = P[s]*exp(-G_TILT*s). Flattens the
# lattice's s-profile so all answer-relevant states fit f32 range; folded into
# the sh1 scalar, the host-built W/end-mask, and the logacc initialization.
G_TILT = 1.75
OFFS = 30.0  # rescale offset: row max is normalized to e^OFFS, not 1

_prog = None  # cached compiled Bass program
_last_results = None


def _build_program():
    from contextlib import ExitStack

    import concourse.bacc as bacc
    import concourse.bass as bass
    import concourse.mybir as mybir
    import concourse.tile as tile

    F32 = mybir.dt.float32
    BF16 = mybir.dt.bfloat16
    OP = mybir.AluOpType
    AF = mybir.ActivationFunctionType
    AX = mybir.AxisListType
    PSUM = bass.MemorySpace.PSUM

    TCH = 128            # time-chunk length
    NCH = T // TCH       # 4 chunks
    NQ = BL // 4         # 32 sample quads per chunk
    NG = NQ // 4         # 8 quad-groups (wg DMA granularity)
    E1 = float(np.exp(-G_TILT))
    OFFE = float(np.exp(OFFS))

    nc = bacc.Bacc("TRN2", target_bir_lowering=False, debug=False)

    # ypt[k,h,c,b,t] = y_pred[b, k*TCH+t, h*128+c] (bf16, host-transposed)
    ypt_d = nc.dram_tensor("ypt", [NCH, 2, 128, BL, TCH], BF16,
                           kind="ExternalInput").ap()
    # wgg[g,c,(q4 si m)]: 4 quads' packed one-hot weights per group
    wgg_d = nc.dram_tensor("wgg", [NG, 128, 4 * 4 * 256], BF16,
                           kind="ExternalInput").ap()
    ybe_d = nc.dram_tensor("ybe", [BL, T], F32, kind="ExternalInput").ap()
    em_d = nc.dram_tensor("em", [BL, S], F32, kind="ExternalInput").ap()
    idf_d = nc.dram_tensor("idf", [128, 128], BF16, kind="ExternalInput").ap()
    pend_d = nc.dram_tensor("pend", [BL, 1], F32, kind="ExternalOutput").ap()
    mxh_d = nc.dram_tensor("mxh", [BL, T // RBLK], F32, kind="ExternalOutput").ap()

    with tile.TileContext(nc) as tc, ExitStack() as ctx:
        # ---- persistent SBUF state (one pool, unique tags) ----
        per = ctx.enter_context(tc.tile_pool(name="per", bufs=1))
        ybe_sb = per.tile([128, T], F32, tag="ybe", name="ybe_sb")
        em_sb = per.tile([128, S], F32, tag="em", name="em_sb")
        idf = per.tile([128, 128], BF16, tag="idf", name="idf_sb")
        pa = per.tile([128, 264], F32, tag="pa", name="pa")
        pb = per.tile([128, 264], F32, tag="pb", name="pb")
        mxh = per.tile([128, T // RBLK], F32, tag="mxh", name="mxh")

        nc.sync.dma_start(ybe_sb[:], ybe_d)
        nc.sync.dma_start(em_sb[:], em_d)
        nc.sync.dma_start(idf[:], idf_d)
        nc.vector.memset(pa[:], 0.0)
        nc.vector.memset(pb[:], 0.0)

        # ---- pools ----
        ypl = ctx.enter_context(tc.tile_pool(name="ypl", bufs=3))
        wpl = ctx.enter_context(tc.tile_pool(name="wpl", bufs=3))
        gcp = ctx.enter_context(tc.tile_pool(name="gcp", bufs=2))
        vpl = ctx.enter_context(tc.tile_pool(name="vpl", bufs=3))
        spl = ctx.enter_context(tc.tile_pool(name="spl", bufs=6))
        gpp = ctx.enter_context(tc.tile_pool(name="gpp", space=PSUM, bufs=3))
        yyp = ctx.enter_context(tc.tile_pool(name="yyp", space=PSUM, bufs=5))

        gc_t = {}   # chunk -> gather SBUF tile [128m, (q si t)] (bf16)
        yp_t = {}   # chunk -> (lo, hi) staged y tiles [128c, (b t)] (bf16)
        wg_t = {}   # (chunk, group) -> weight tile [128c, (q4 si m)]

        def ypt_load(k):
            lo = ypl.tile([128, BL * TCH], BF16, tag="ypt", name="yplo")
            hi = ypl.tile([128, BL * TCH], BF16, tag="ypt", name="yphi")
            nc.sync.dma_start(lo[:], ypt_d[k, 0].rearrange("c b t -> c (b t)"))
            nc.sync.dma_start(hi[:], ypt_d[k, 1].rearrange("c b t -> c (b t)"))
            yp_t[k] = (lo, hi)

        def gather_open(k):
            g = gcp.tile([128, TCH * 128], BF16, tag="gc", name="gc")
            gc_t[k] = g

        def wg_load(k, grp):
            w = wpl.tile([128, 4 * 4 * 256], BF16, tag="w", name="w")
            nc.sync.dma_start(w[:], wgg_d[grp])
            wg_t[(k, grp)] = w

        gq_t = {}  # chunk -> current quad's PSUM tile

        def gather_si(k, q, si):
            # one sample's pair of matmuls; quad's PSUM tile opens at si=0
            # and is staged to SBUF (with +EPS via ACT bias) at si=3
            if si == 0:
                gq_t[k] = gpp.tile([128, 512], F32, tag="gq", name="gq")
            gq = gq_t[k]
            w4 = wg_t[(k, q // 4)][:].rearrange(
                "c (q4 si m) -> c q4 si m", q4=4, si=4)[:, q % 4]
            lo, hi = yp_t[k]
            smp = q * 4 + si
            sl = slice(si * 128, (si + 1) * 128)
            ysl = slice(smp * TCH, (smp + 1) * TCH)
            nc.tensor.matmul(gq[:, sl], w4[:, si, 0:128], lo[:, ysl],
                             start=True, stop=False)
            nc.tensor.matmul(gq[:, sl], w4[:, si, 128:256], hi[:, ysl],
                             start=False, stop=True)
            if si == 3:
                nc.scalar.activation(gc_t[k][:, q * 512:(q + 1) * 512], gq[:],
                                     AF.Copy, bias=EPS)

        def gather_quad(k, q):
            for si in range(4):
                gather_si(k, q, si)

        def yy_view(k, tl):
            # [128m, 128b] view of gc_t[k] at time tl: b = q*4+si,
            # element (q,si) at col q*512 + si*128 + tl
            g = gc_t[k][:]
            return bass.AP(g.tensor, g.offset + tl,
                           [g.ap[0], [512, 32], [128, 4]])

        AOFF = 134  # A[s] lives at col AOFF+s of the *current* state tensor

        def dp_step(t, pcur, pnxt, rec2):
            k, tl = divmod(t, TCH)
            yy = yyp.tile([128, 128], BF16, tag="yy")
            nc.tensor.transpose(yy[:], yy_view(k, tl), idf[:])
            # A[s] = P[s] + e^-g*P[s-1], written into pcur's scratch region
            nc.vector.scalar_tensor_tensor(pcur[:, AOFF:AOFF + 129],
                                           pcur[:, 0:129], E1,
                                           pcur[:, 1:130], OP.mult, OP.add)
            u3 = pnxt[:, 1:131].rearrange("p (s two) -> p s two", two=2)
            a_even = pcur[:, AOFF:AOFF + 130].rearrange(
                "p (s two) -> p s two", two=2)[:, :, 0]
            # even states: (A_even * ybe) [* rec2 on post-rescale steps]
            if rec2 is None:
                nc.vector.tensor_scalar(u3[:, :, 0], a_even, ybe_sb[:, t:t + 1],
                                        None, OP.mult)
            else:
                nc.vector.tensor_scalar(u3[:, :, 0], a_even, ybe_sb[:, t:t + 1],
                                        rec2[:], OP.mult, OP.mult)
            # one 2D-strided multiply covers skip & label terms:
            #   X[0,l] = P[2l]     * yy[0..63]   (skip: e^-2g * masked onehot)
            #   X[1,l] = A[2l+1]   * yy[64..127] (label emission)
            stz = bass.AP(pcur[:].tensor, pcur[:].offset,
                          [pcur[:].ap[0], [AOFF + 1, 2], [2, 64]])
            x = vpl.tile([128, 128], F32, tag="x")
            if rec2 is None:
                nc.vector.tensor_tensor(x[:], stz, yy[:], OP.mult)
            else:
                nc.vector.scalar_tensor_tensor(x[:], stz, rec2[:], yy[:],
                                               OP.mult, OP.mult)
            nc.vector.tensor_tensor(u3[:, 0:64, 1], x[:, 0:64], x[:, 64:128],
                                    OP.add)
            if t % RBLK == RBLK - 1:
                ridx = t // RBLK
                mxc = mxh[:, ridx:ridx + 1]
                nc.vector.tensor_reduce(mxc, pnxt[:, 1:130], AX.X, OP.max)
                rec = spl.tile([128, 1], F32, tag="rec")
                nc.vector.reciprocal(rec[:], mxc)
                rec2n = spl.tile([128, 1], F32, tag="rec2")
                nc.vector.tensor_scalar(rec2n[:], rec[:], OFFE, None, OP.mult)
                return rec2n
            return None

        # chunk 0: load + gather before the DP loop starts
        ypt_load(0)
        gather_open(0)
        for grp in range(NG):
            wg_load(0, grp)
        for q in range(NQ):
            gather_quad(0, q)

        # init (t = 0): P[s=0] = ybe[:,0]; P~[s=1] = e^-g * y_lab(l=0,t=0)
        yy0 = yyp.tile([128, 128], BF16, tag="yy")
        nc.tensor.transpose(yy0[:], yy_view(0, 0), idf[:])
        nc.vector.tensor_copy(pa[:, 1:2], ybe_sb[:, 0:1])
        nc.vector.tensor_scalar(pa[:, 2:3], yy0[:, 64:65], E1, None, OP.mult)

        pcur, pnxt = pa, pb
        rec2 = None
        for t in range(1, T):
            k, tl = divmod(t, TCH)
            # interleave next-chunk gather emission through this chunk's DP
            # steps (one sample's 2 matmuls per step) so the PE never falls
            # far behind on the per-step yy transposes
            if k + 1 < NCH:
                if k == 0:
                    # chunk 0 has 127 DP steps; pack 2 samples per step
                    if tl == 1:
                        ypt_load(1)
                        gather_open(1)
                    if tl % 8 == 1 and tl <= 57:
                        wg_load(1, tl // 8)
                    if tl <= 64:
                        u = 2 * (tl - 1)
                        gather_si(1, u // 4, u % 4)
                        gather_si(1, (u + 1) // 4, (u + 1) % 4)
                else:
                    if tl == 0:
                        ypt_load(k + 1)
                        gather_open(k + 1)
                    if tl % 16 == 0:
                        wg_load(k + 1, tl // 16)
                    gather_si(k + 1, tl // 4, tl % 4)
            rec2 = dp_step(t, pcur, pnxt, rec2)
            pcur, pnxt = pnxt, pcur
        if rec2 is not None:
            # the last rescale's scaling never got absorbed; apply it now
            nc.vector.tensor_scalar_mul(pcur[:, 1:130], pcur[:, 1:130], rec2[:])

        # final: export pend = sum(P * endmask) and the rescale history;
        # the exact logs happen on the host.
        scre = per.tile([128, S], F32, tag="scre", name="scre")
        nc.vector.tensor_tensor(scre[:], pcur[:, 1:130], em_sb[:], OP.mult)
        pend = per.tile([128, 1], F32, tag="pend", name="pend")
        nc.vector.tensor_reduce(pend[:], scre[:], AX.X, OP.add)
        nc.sync.dma_start(pend_d, pend[:])
        nc.sync.dma_start(mxh_d, mxh[:])

    nc.compile()
    return nc


def _host_derived(y_true, y_pred, label_length):
    import ml_dtypes

    lab = np.asarray(y_true, dtype=np.int64)  # [B, 64]
    llv = np.asarray(label_length).reshape(-1)
    # packed one-hots: [B, C, 128]; cols 0..63 skip-masked labels scaled by
    # e^(-2g), cols 64..127 labels (validity-masked)
    vm = (np.arange(L)[None, :] < llv[:, None])  # valid odd state s=2l+1
    zm = np.concatenate([np.zeros((B, 1), bool), lab[:, 1:] != lab[:, :-1]], axis=1)
    w = np.zeros((B, C, 128), dtype=np.float32)
    bb = np.repeat(np.arange(B), L)
    ll = np.tile(np.arange(L), B)
    cc = lab.reshape(-1)
    w[bb, cc, L + ll] = vm.reshape(-1).astype(np.float32)
    w[bb, cc, ll] = np.where(
        (zm & vm).reshape(-1),
        np.float32(np.exp(-2.0 * G_TILT)),
        w[bb, cc, ll],
    )
    # device layout: [group, 128c(lo), 4q, 4si, (ck m)] with c = ck*128 + c_lo
    w6 = w.reshape(B // 16, 4, 4, 2, 128, 128)      # [g, q4, si, ck, c_lo, m]
    w6 = w6.transpose(0, 4, 1, 2, 3, 5)             # [g, c_lo, q4, si, ck, m]
    wgg = np.ascontiguousarray(
        w6.reshape(B // 16, 128, 4 * 4 * 256).astype(ml_dtypes.bfloat16)
    )
    ybe = np.ascontiguousarray(np.asarray(y_pred)[:, :, C - 1] + np.float32(EPS))
    return wgg, ybe


def kernel(y_true, y_pred, input_length, label_length, _trace=False):
    global _prog, _last_results
    from concourse.bass_utils import run_bass_kernel_spmd

    y_true = np.asarray(y_true)
    import ml_dtypes
    y_pred = np.asarray(y_pred, dtype=np.float32)
    y_pred_bf = y_pred.astype(ml_dtypes.bfloat16)
    label_length = np.asarray(label_length).reshape(-1)

    TCH = 128
    NCH = T // TCH
    # ypt[core][k,h,c,b,t] = y_pred[b, k*TCH+t, h*128+c]
    ypt = y_pred_bf.reshape(NCORES, BL, NCH, TCH, 2, 128).transpose(
        0, 2, 4, 5, 1, 3)
    ypt = np.ascontiguousarray(ypt)

    wgg, ybe = _host_derived(y_true, y_pred, label_length)
    em = np.zeros((B, S), dtype=np.float32)
    bidx = np.arange(B)
    em[bidx, 2 * label_length] = 1.0
    em[bidx, 2 * label_length - 1] = np.float32(np.exp(-G_TILT))
    idf = np.eye(128, dtype=ml_dtypes.bfloat16)

    if _prog is None:
        _prog = _build_program()

    NGC = BL // 16  # wg groups per core
    in_maps = []
    for i in range(NCORES):
        sl = slice(i * BL, (i + 1) * BL)
        slg = slice(i * NGC, (i + 1) * NGC)
        in_maps.append({
            "ypt": ypt[i],
            "wgg": wgg[slg],
            "ybe": ybe[sl],
            "em": em[sl],
            "idf": idf,
        })
    res = run_bass_kernel_spmd(_prog, in_maps, core_ids=list(range(NCORES)),
                               trace=_trace)
    _last_results = res
    pend = np.concatenate([r["pend"] for r in res.results], axis=0).reshape(-1)
    mxh = np.concatenate([r["mxh"] for r in res.results], axis=0)
    nres = mxh.shape[1]
    logacc = np.log(mxh.astype(np.float64)).sum(axis=1) - OFFS * nres
    loss = -(np.log(pend.astype(np.float64)) + logacc
             + G_TILT * 2.0 * label_length.astype(np.float64))
    return loss.reshape(B, 1).astype(np.float32)


if __name__ == "__main__":
    rng = np.random.default_rng(0)
    yp = rng.random((B, T, C), dtype=np.float32)
    yp /= yp.sum(-1, keepdims=True)
    yt = rng.integers(0, C - 1, size=(B, L)).astype(np.int32)
    il = np.full((B, 1), T, dtype=np.int32)
    ll = rng.integers(32, L + 1, size=(B, 1)).astype(np.int32)
    print(kernel(yt, yp, il, ll)[:4])


# revision 24
# speedup vs baseline: 1.0909x; 1.0480x over previous
"""CTC loss (keras ctc_batch_cost semantics) on 8 Trainium2 NeuronCores.

Strategy (pure data parallelism, batch sharded 128 samples/core):
  - Blank-normalized DP in probability space: R[t,s] = P[t,s]/prod(ybe)
    with ybe[b,t] = y_pred[b,t,blank]+EPS. The device computes
        R'[2l]   = R[2l] + e^-g R[2l-1]                       (free!)
        R'[2l+1] = yl*(R[2l+1] + e^-g R[2l]) + ys*R[2l-1]
    where yl/ys are blank-normalized label/skip emissions
    yhat = (y+EPS)/ybe gathered per (sample,t) on the PE. The cumulative
    blank product re-enters the loss on the host as sum(log ybe) (f64).
  - 3 DVE ops per step: a single fused state tile holds both the pcur
    and pnxt regions so one 2D-strided access spans A (in pnxt) and
    R[2l-1] (in pcur); DVE instruction count is what the serial chain
    pays for, so ops are minimized.
  - y_pred is transposed on the HOST to [chunk, c_half, c, b, t]; each
    (chunk, half) is one 4MB DMA with 32KB contiguous partition lines.
  - Emission gather: per-sample one-hot matmuls G[b] = W[b].T @ yhatT[b]
    (cols 0..63 skip-masked e^-2g one-hots, 64..127 label one-hots),
    staged PSUM->SBUF by ACT, then a per-step PE transpose feeds the DVE
    a [128b, 128m] tile from PSUM. Chunk 0 is gathered in two FD=64
    halves so the DP starts after half the gather latency.
  - Rescale every RBLK steps: row max -> mxh, next step pre-scales the
    state in place by e^OFFS/max. Exact logs happen on the host.
  - Static per-state tilt e^(-G_TILT*s) keeps the lattice in f32 range.
"""

import numpy as np

B, T, C, L = 1024, 512, 256, 64
S = 2 * L + 1  # 129
NCORES = 8
BL = B // NCORES  # 128 samples per core
EPS = 1e-7
RBLK = 12            # rescale period (time steps)
NRES = 42            # number of rescales: t in {11, 23, ..., 503}
G_TILT = 1.75
OFFS = 15.0          # rescale offset: row max is normalized to e^OFFS

_prog = None  # cached compiled Bass program
_last_results = None


def _build_program():
    from contextlib import ExitStack

    import concourse.bacc as bacc
    import concourse.bass as bass
    import concourse.mybir as mybir
    import concourse.tile as tile

    F32 = mybir.dt.float32
    BF16 = mybir.dt.bfloat16
    OP = mybir.AluOpType
    AF = mybir.ActivationFunctionType
    AX = mybir.AxisListType
    PSUM = bass.MemorySpace.PSUM

    TCH = 128            # time-chunk length
    NCH = T // TCH       # 4 chunks
    NQ = BL // 4         # 32 sample quads per chunk
    NG = NQ // 4         # 8 quad-groups (wg DMA granularity)
    E1 = float(np.exp(-G_TILT))
    OFFE = float(np.exp(OFFS))
    PB = 136             # pnxt region base offset inside the fused state tile

    nc = bacc.Bacc("TRN2", target_bir_lowering=False, debug=False)

    # ypt[k,h,c,b,t] = yhat[b, k*TCH+t, h*128+c] (bf16, host-transposed)
    ypt_d = nc.dram_tensor("ypt", [NCH, 2, 128, BL, TCH], BF16,
                           kind="ExternalInput").ap()
    # wgg[g,c,(q4 si m)]: 4 quads' packed one-hot weights per group
    wgg_d = nc.dram_tensor("wgg", [NG, 128, 4 * 4 * 256], BF16,
                           kind="ExternalInput").ap()
    em_d = nc.dram_tensor("em", [BL, S], F32, kind="ExternalInput").ap()
    idf_d = nc.dram_tensor("idf", [128, 128], BF16, kind="ExternalInput").ap()
    pend_d = nc.dram_tensor("pend", [BL, 1], F32, kind="ExternalOutput").ap()
    mxh_d = nc.dram_tensor("mxh", [BL, NRES], F32, kind="ExternalOutput").ap()

    with tile.TileContext(nc) as tc, ExitStack() as ctx:
        # ---- persistent SBUF state (one pool, unique tags) ----
        per = ctx.enter_context(tc.tile_pool(name="per", bufs=1))
        em_sb = per.tile([128, S], F32, tag="em", name="em_sb")
        idf = per.tile([128, 128], BF16, tag="idf", name="idf_sb")
        # fused state tile: pcur/pnxt regions at col 0 / col PB, col 0 of
        # each region is a zero pad (R[s] lives at col base+1+s)
        pab = per.tile([128, 2 * PB], F32, tag="pab", name="pab")
        mxh = per.tile([128, NRES], F32, tag="mxh", name="mxh")
        g0a = per.tile([128, 32 * 256], BF16, tag="gc0a", name="gc0a")
        g0b = per.tile([128, 32 * 256], BF16, tag="gc0b", name="gc0b")

        nc.sync.dma_start(em_sb[:], em_d)
        nc.sync.dma_start(idf[:], idf_d)
        nc.vector.memset(pab[:], 0.0)

        # ---- pools ----
        ypl = ctx.enter_context(tc.tile_pool(name="ypl", bufs=2))
        wpl = ctx.enter_context(tc.tile_pool(name="wpl", bufs=2))
        gcp = ctx.enter_context(tc.tile_pool(name="gcp", bufs=2))
        vpl = ctx.enter_context(tc.tile_pool(name="vpl", bufs=3))
        spl = ctx.enter_context(tc.tile_pool(name="spl", bufs=6))
        gpp = ctx.enter_context(tc.tile_pool(name="gpp", space=PSUM, bufs=3))
        yyp = ctx.enter_context(tc.tile_pool(name="yyp", space=PSUM, bufs=5))

        gc_t = {}   # chunk -> gather SBUF tile(s)
        yp_t = {}   # chunk -> {half: staged yhat tile [128c, (b t)]}
        wg_t = {}   # (chunk, group) -> weight tile [128c, (q4 si m)]
        gq_t = {}   # (chunk, sub) -> current quad's PSUM tile

        def ypt_load_half(k, h):
            tl_ = ypl.tile([128, BL * TCH], BF16, tag="ypt", name="ypth")
            nc.sync.dma_start(tl_[:], ypt_d[k, h].rearrange("c b t -> c (b t)"))
            yp_t.setdefault(k, {})[h] = tl_

        def ypt_load(k):
            ypt_load_half(k, 0)
            ypt_load_half(k, 1)

        def wg_load(k, grp, phase=0):
            w = wpl.tile([128, 4 * 4 * 256], BF16, tag="w", name="w")
            nc.sync.dma_start(w[:], wgg_d[grp])
            wg_t[(k, grp, phase)] = w

        def gather_si(k, q, si, sub=None):
            # one sample's pair of matmuls; the quad's PSUM tile opens at
            # si=0 and is staged to SBUF at si=3. sub: chunk-0 FD=64 halves.
            fd = TCH if sub is None else TCH // 2
            if si == 0:
                gq_t[(k, sub)] = gpp.tile([128, 512], F32, tag="gq",
                                          name="gq")
            gq = gq_t[(k, sub)]
            w4 = wg_t[(k, q // 4, 1 if sub == 1 else 0)][:].rearrange(
                "c (q4 si m) -> c q4 si m", q4=4, si=4)[:, q % 4]
            lo, hi = yp_t[k][0], yp_t[k][1]
            smp = q * 4 + si
            t0 = smp * TCH + (0 if sub in (None, 0) else fd)
            sl = slice(si * fd, (si + 1) * fd)
            nc.tensor.matmul(gq[:, sl], w4[:, si, 0:128], lo[:, t0:t0 + fd],
                             start=True, stop=False)
            nc.tensor.matmul(gq[:, sl], w4[:, si, 128:256], hi[:, t0:t0 + fd],
                             start=False, stop=True)
            if si == 3:
                g = gc_t[k] if sub is None else gc_t[k][sub]
                nc.scalar.activation(g[:, q * 4 * fd:(q + 1) * 4 * fd],
                                     gq[:, 0:4 * fd], AF.Copy, bias=0.0)

        def yy_view(k, tl):
            # [128m, 128b] view of the gather tile at time tl; b = q*4+si
            if k == 0:
                g = gc_t[0][0 if tl < 64 else 1][:]
                return bass.AP(g.tensor, g.offset + tl % 64,
                               [g.ap[0], [256, 32], [64, 4]])
            g = gc_t[k][:]
            return bass.AP(g.tensor, g.offset + tl,
                           [g.ap[0], [512, 32], [128, 4]])

        def dp_step(t, cb, nb, rec2):
            # cb/nb: column bases of the pcur/pnxt regions inside pab
            k, tl = divmod(t, TCH)
            yy = yyp.tile([128, 128], BF16, tag="yy")
            nc.tensor.transpose(yy[:], yy_view(k, tl), idf[:])
            if rec2 is not None:
                # absorb last rescale: pcur *= e^OFFS/max, in place
                nc.vector.tensor_scalar_mul(pab[:, cb + 1:cb + 130],
                                            pab[:, cb + 1:cb + 130], rec2[:])
            # A[s] = R[s] + e^-g R[s-1] -> pnxt (even states final)
            nc.vector.scalar_tensor_tensor(pab[:, nb + 1:nb + 130],
                                           pab[:, cb:cb + 129], E1,
                                           pab[:, cb + 1:cb + 130],
                                           OP.mult, OP.add)
            # x[0,l] = R[2l-1] * yy_skip[l];  x[1,l] = A[2l+1] * yy_lab[l]
            stz = bass.AP(pab[:].tensor, pab[:].offset + cb,
                          [pab[:].ap[0], [nb - cb + 2, 2], [2, 64]])
            x = vpl.tile([128, 128], F32, tag="x")
            nc.vector.tensor_tensor(x[:], stz, yy[:], OP.mult)
            # odd states: overwrite A_odd with yl*A + ys*R[2l-1]
            odd = bass.AP(pab[:].tensor, pab[:].offset + nb + 2,
                          [pab[:].ap[0], [2, 64]])
            nc.vector.tensor_tensor(odd, x[:, 0:64], x[:, 64:128], OP.add)
            if t % RBLK == RBLK - 1 and t < NRES * RBLK:
                ridx = t // RBLK
                mxc = mxh[:, ridx:ridx + 1]
                nc.vector.tensor_reduce(mxc, pab[:, nb + 1:nb + 130],
                                        AX.X, OP.max)
                rec = spl.tile([128, 1], F32, tag="rec")
                nc.vector.reciprocal(rec[:], mxc)
                rec2n = spl.tile([128, 1], F32, tag="rec2")
                nc.vector.tensor_scalar(rec2n[:], rec[:], OFFE, None, OP.mult)
                return rec2n
            return None

        # ---- chunk 0: load + gather first FD=64 half before the DP ----
        ypt_load(0)
        gc_t[0] = (g0a, g0b)
        for grp in range(NG):
            wg_load(0, grp)
        for q in range(NQ):
            for si in range(4):
                gather_si(0, q, si, sub=0)

        # init (t = 0): R[s=0] = 1; R~[s=1] = e^-g * yhat_lab(l=0,t=0)
        yy0 = yyp.tile([128, 128], BF16, tag="yy")
        nc.tensor.transpose(yy0[:], yy_view(0, 0), idf[:])
        nc.vector.memset(pab[:, 1:2], 1.0)
        nc.vector.tensor_scalar(pab[:, 2:3], yy0[:, 64:65], E1, None, OP.mult)

        cb, nb = 0, PB
        rec2 = None
        for t in range(1, T):
            k, tl = divmod(t, TCH)
            # interleave gather work through the DP steps so the PE keeps
            # just ahead of the per-step yy transposes
            if k == 0:
                # steps 1..43: chunk-0 second half (3 samples/step);
                # steps 64..127: chunk 1 (2 samples/step). Chunk-1 y tiles
                # stream in as the chunk-0 tiles free up (ypl bufs=3).
                if tl <= 43:
                    w1s = {0: 0, 5: 1, 10: 2, 16: 3, 21: 4, 26: 5, 32: 6, 37: 7}
                    if tl - 1 in w1s:
                        wg_load(0, w1s[tl - 1], phase=1)
                    for j in range(3):
                        u = 3 * (tl - 1) + j
                        if u < 128:
                            gather_si(0, u // 4, u % 4, sub=1)
                if tl == 44:
                    ypt_load(1)
                    g = gcp.tile([128, TCH * 128], BF16, tag="gc", name="gc")
                    gc_t[1] = g
                if 64 <= tl <= 127:
                    if (tl - 64) % 8 == 0:
                        wg_load(1, (tl - 64) // 8)
                    u = 2 * (tl - 64)
                    gather_si(1, u // 4, u % 4)
                    gather_si(1, (u + 1) // 4, (u + 1) % 4)
            elif k + 1 < NCH:
                if tl == 0:
                    ypt_load(k + 1)
                    g = gcp.tile([128, TCH * 128], BF16, tag="gc", name="gc")
                    gc_t[k + 1] = g
                if tl % 16 == 0:
                    wg_load(k + 1, tl // 16)
                gather_si(k + 1, tl // 4, tl % 4)
            rec2 = dp_step(t, cb, nb, rec2)
            cb, nb = nb, cb
        if rec2 is not None:
            nc.vector.tensor_scalar_mul(pab[:, cb + 1:cb + 130],
                                        pab[:, cb + 1:cb + 130], rec2[:])

        # final: pend = sum(R * endmask); exact logs happen on the host
        scre = per.tile([128, S], F32, tag="scre", name="scre")
        nc.vector.tensor_tensor(scre[:], pab[:, cb + 1:cb + 130], em_sb[:],
                                OP.mult)
        pend = per.tile([128, 1], F32, tag="pend", name="pend")
        nc.vector.tensor_reduce(pend[:], scre[:], AX.X, OP.add)
        nc.sync.dma_start(pend_d, pend[:])
        nc.sync.dma_start(mxh_d, mxh[:])

    nc.compile()
    return nc


def _host_derived(y_true, label_length):
    import ml_dtypes

    lab = np.asarray(y_true, dtype=np.int64)  # [B, 64]
    llv = np.asarray(label_length).reshape(-1)
    # packed one-hots: [B, C, 128]; cols 0..63 skip-masked labels scaled by
    # e^(-2g), cols 64..127 labels (validity-masked)
    vm = (np.arange(L)[None, :] < llv[:, None])  # valid odd state s=2l+1
    zm = np.concatenate([np.zeros((B, 1), bool), lab[:, 1:] != lab[:, :-1]], axis=1)
    w = np.zeros((B, C, 128), dtype=np.float32)
    bb = np.repeat(np.arange(B), L)
    ll = np.tile(np.arange(L), B)
    cc = lab.reshape(-1)
    w[bb, cc, L + ll] = vm.reshape(-1).astype(np.float32)
    w[bb, cc, ll] = np.where(
        (zm & vm).reshape(-1),
        np.float32(np.exp(-2.0 * G_TILT)),
        w[bb, cc, ll],
    )
    # device layout: [group, 128c(lo), 4q, 4si, (ck m)] with c = ck*128 + c_lo
    w6 = w.reshape(B // 16, 4, 4, 2, 128, 128)      # [g, q4, si, ck, c_lo, m]
    w6 = w6.transpose(0, 4, 1, 2, 3, 5)             # [g, c_lo, q4, si, ck, m]
    wgg = np.ascontiguousarray(
        w6.reshape(B // 16, 128, 4 * 4 * 256).astype(ml_dtypes.bfloat16)
    )
    return wgg


def kernel(y_true, y_pred, input_length, label_length, _trace=False):
    global _prog, _last_results
    from concourse.bass_utils import run_bass_kernel_spmd

    y_true = np.asarray(y_true)
    import ml_dtypes
    y_pred = np.asarray(y_pred, dtype=np.float32)
    label_length = np.asarray(label_length).reshape(-1)

    # blank-normalized emissions; the blank product returns via logbeta
    ybe = y_pred[:, :, C - 1] + np.float32(EPS)          # [B, T]
    yhat = (y_pred + np.float32(EPS)) / ybe[:, :, None]
    yhat_bf = yhat.astype(ml_dtypes.bfloat16)
    logbeta = np.log(ybe.astype(np.float64)).sum(axis=1)  # [B]

    TCH = 128
    NCH = T // TCH
    # ypt[core][k,h,c,b,t] = yhat[b, k*TCH+t, h*128+c]
    ypt = yhat_bf.reshape(NCORES, BL, NCH, TCH, 2, 128).transpose(
        0, 2, 4, 5, 1, 3)
    ypt = np.ascontiguousarray(ypt)

    wgg = _host_derived(y_true, label_length)
    em = np.zeros((B, S), dtype=np.float32)
    bidx = np.arange(B)
    em[bidx, 2 * label_length] = 1.0
    em[bidx, 2 * label_length - 1] = np.float32(np.exp(-G_TILT))
    idf = np.eye(128, dtype=ml_dtypes.bfloat16)

    if _prog is None:
        _prog = _build_program()

    NGC = BL // 16  # wg groups per core
    in_maps = []
    for i in range(NCORES):
        sl = slice(i * BL, (i + 1) * BL)
        slg = slice(i * NGC, (i + 1) * NGC)
        in_maps.append({
            "ypt": ypt[i],
            "wgg": wgg[slg],
            "em": em[sl],
            "idf": idf,
        })
    res = run_bass_kernel_spmd(_prog, in_maps, core_ids=list(range(NCORES)),
                               trace=_trace)
    _last_results = res
    pend = np.concatenate([r["pend"] for r in res.results], axis=0).reshape(-1)
    mxh = np.concatenate([r["mxh"] for r in res.results], axis=0)
    logacc = np.log(mxh.astype(np.float64)).sum(axis=1) - OFFS * NRES
    loss = -(np.log(pend.astype(np.float64)) + logacc + logbeta
             + G_TILT * 2.0 * label_length.astype(np.float64))
    return loss.reshape(B, 1).astype(np.float32)


if __name__ == "__main__":
    rng = np.random.default_rng(0)
    yp = rng.random((B, T, C), dtype=np.float32)
    yp /= yp.sum(-1, keepdims=True)
    yt = rng.integers(0, C - 1, size=(B, L)).astype(np.int32)
    il = np.full((B, 1), T, dtype=np.int32)
    ll = rng.integers(32, L + 1, size=(B, 1)).astype(np.int32)
    print(kernel(yt, yp, il, ll)[:4])


# revision 26
# speedup vs baseline: 1.0925x; 1.0015x over previous
"""CTC loss (keras ctc_batch_cost semantics) on 8 Trainium2 NeuronCores.

Strategy (pure data parallelism, batch sharded 128 samples/core):
  - Blank-normalized DP in probability space: R[t,s] = P[t,s]/prod(ybe)
    with ybe[b,t] = y_pred[b,t,blank]+EPS. The device computes
        R'[2l]   = R[2l] + e^-g R[2l-1]                       (free!)
        R'[2l+1] = yl*(R[2l+1] + e^-g R[2l]) + ys*R[2l-1]
    where yl/ys are blank-normalized label/skip emissions
    yhat = (y+EPS)/ybe gathered per (sample,t) on the PE. The cumulative
    blank product re-enters the loss on the host as sum(log ybe) (f64).
  - 3 DVE ops per step: a single fused state tile holds both the pcur
    and pnxt regions so one 2D-strided access spans A (in pnxt) and
    R[2l-1] (in pcur); DVE instruction count is what the serial chain
    pays for, so ops are minimized.
  - y_pred is transposed on the HOST to [chunk, c_half, c, b, t]; each
    (chunk, half) is one 4MB DMA with 32KB contiguous partition lines.
  - Emission gather: per-sample one-hot matmuls G[b] = W[b].T @ yhatT[b]
    (cols 0..63 skip-masked e^-2g one-hots, 64..127 label one-hots),
    staged PSUM->SBUF by ACT, then a per-step PE transpose feeds the DVE
    a [128b, 128m] tile from PSUM. Chunk 0 is gathered in two FD=64
    halves so the DP starts after half the gather latency.
  - Rescale every RBLK steps: row max -> mxh, next step pre-scales the
    state in place by e^OFFS/max. Exact logs happen on the host.
  - Static per-state tilt e^(-G_TILT*s) keeps the lattice in f32 range.
"""

import numpy as np

B, T, C, L = 1024, 512, 256, 64
S = 2 * L + 1  # 129
NCORES = 8
BL = B // NCORES  # 128 samples per core
EPS = 1e-7
RBLK = 12            # rescale period (time steps)
NRES = 42            # number of rescales: t in {11, 23, ..., 503}
G_TILT = 1.75
OFFS = 15.0          # rescale offset: row max is normalized to e^OFFS

_prog = None  # cached compiled Bass program
_last_results = None


def _build_program():
    from contextlib import ExitStack

    import concourse.bacc as bacc
    import concourse.bass as bass
    import concourse.mybir as mybir
    import concourse.tile as tile

    F32 = mybir.dt.float32
    BF16 = mybir.dt.bfloat16
    OP = mybir.AluOpType
    AF = mybir.ActivationFunctionType
    AX = mybir.AxisListType
    PSUM = bass.MemorySpace.PSUM

    TCH = 128            # time-chunk length
    NCH = T // TCH       # 4 chunks
    NQ = BL // 4         # 32 sample quads per chunk
    NG = NQ // 4         # 8 quad-groups (wg DMA granularity)
    E1 = float(np.exp(-G_TILT))
    OFFE = float(np.exp(OFFS))
    PB = 136             # pnxt region base offset inside the fused state tile

    nc = bacc.Bacc("TRN2", target_bir_lowering=False, debug=False)

    # ypt[k,h,c,b,t] = yhat[b, k*TCH+t, h*128+c] (bf16, host-transposed)
    ypt_d = nc.dram_tensor("ypt", [NCH, 2, 128, BL, TCH], BF16,
                           kind="ExternalInput").ap()
    # wgg[g,c,(q4 si m)]: 4 quads' packed one-hot weights per group
    wgg_d = nc.dram_tensor("wgg", [NG, 128, 4 * 4 * 256], BF16,
                           kind="ExternalInput").ap()
    em_d = nc.dram_tensor("em", [BL, S], F32, kind="ExternalInput").ap()
    idf_d = nc.dram_tensor("idf", [128, 128], BF16, kind="ExternalInput").ap()
    pend_d = nc.dram_tensor("pend", [BL, 1], F32, kind="ExternalOutput").ap()
    mxh_d = nc.dram_tensor("mxh", [BL, NRES], F32, kind="ExternalOutput").ap()

    with tile.TileContext(nc) as tc, ExitStack() as ctx:
        # ---- persistent SBUF state (one pool, unique tags) ----
        per = ctx.enter_context(tc.tile_pool(name="per", bufs=1))
        em_sb = per.tile([128, S], F32, tag="em", name="em_sb")
        idf = per.tile([128, 128], BF16, tag="idf", name="idf_sb")
        # fused state tile: pcur/pnxt regions at col 0 / col PB, col 0 of
        # each region is a zero pad (R[s] lives at col base+1+s)
        pab = per.tile([128, 2 * PB], F32, tag="pab", name="pab")
        mxh = per.tile([128, NRES], F32, tag="mxh", name="mxh")
        g0a = per.tile([128, 32 * 256], BF16, tag="gc0a", name="gc0a")
        g0b = per.tile([128, 32 * 256], BF16, tag="gc0b", name="gc0b")

        nc.sync.dma_start(em_sb[:], em_d)
        nc.sync.dma_start(idf[:], idf_d)
        nc.vector.memset(pab[:], 0.0)

        # ---- pools ----
        ypl = ctx.enter_context(tc.tile_pool(name="ypl", bufs=2))
        wpl = ctx.enter_context(tc.tile_pool(name="wpl", bufs=2))
        gcp = ctx.enter_context(tc.tile_pool(name="gcp", bufs=2))
        vpl = ctx.enter_context(tc.tile_pool(name="vpl", bufs=3))
        spl = ctx.enter_context(tc.tile_pool(name="spl", bufs=6))
        gpp = ctx.enter_context(tc.tile_pool(name="gpp", space=PSUM, bufs=3))
        yyp = ctx.enter_context(tc.tile_pool(name="yyp", space=PSUM, bufs=5))

        gc_t = {}   # chunk -> gather SBUF tile(s)
        yp_t = {}   # chunk -> {half: staged yhat tile [128c, (b t)]}
        wg_t = {}   # (chunk, group) -> weight tile [128c, (q4 si m)]
        gq_t = {}   # (chunk, sub) -> current quad's PSUM tile

        def ypt_load_half(k, h):
            tl_ = ypl.tile([128, BL * TCH], BF16, tag="ypt", name="ypth")
            nc.sync.dma_start(tl_[:], ypt_d[k, h].rearrange("c b t -> c (b t)"))
            yp_t.setdefault(k, {})[h] = tl_

        def ypt_load(k):
            ypt_load_half(k, 0)
            ypt_load_half(k, 1)

        def wg_load(k, grp, phase=0):
            w = wpl.tile([128, 4 * 4 * 256], BF16, tag="w", name="w")
            nc.sync.dma_start(w[:], wgg_d[grp])
            wg_t[(k, grp, phase)] = w

        def gather_si(k, q, si, sub=None):
            # one sample's pair of matmuls; the quad's PSUM tile opens at
            # si=0 and is staged to SBUF at si=3. sub: chunk-0 FD=64 halves.
            fd = TCH if sub is None else TCH // 2
            if si == 0:
                gq_t[(k, sub)] = gpp.tile([128, 512], F32, tag="gq",
                                          name="gq")
            gq = gq_t[(k, sub)]
            w4 = wg_t[(k, q // 4, 0)][:].rearrange(
                "c (q4 si m) -> c q4 si m", q4=4, si=4)[:, q % 4]
            lo, hi = yp_t[k][0], yp_t[k][1]
            smp = q * 4 + si
            t0 = smp * TCH + (0 if sub in (None, 0) else fd)
            sl = slice(si * fd, (si + 1) * fd)
            nc.tensor.matmul(gq[:, sl], w4[:, si, 0:128], lo[:, t0:t0 + fd],
                             start=True, stop=False)
            nc.tensor.matmul(gq[:, sl], w4[:, si, 128:256], hi[:, t0:t0 + fd],
                             start=False, stop=True)
            if si == 3:
                g = gc_t[k] if sub is None else gc_t[k][sub]
                nc.scalar.activation(g[:, q * 4 * fd:(q + 1) * 4 * fd],
                                     gq[:, 0:4 * fd], AF.Copy, bias=0.0)

        def yy_view(k, tl):
            # [128m, 128b] view of the gather tile at time tl; b = q*4+si
            if k == 0:
                g = gc_t[0][0 if tl < 64 else 1][:]
                return bass.AP(g.tensor, g.offset + tl % 64,
                               [g.ap[0], [256, 32], [64, 4]])
            g = gc_t[k][:]
            return bass.AP(g.tensor, g.offset + tl,
                           [g.ap[0], [512, 32], [128, 4]])

        def dp_step(t, cb, nb, rec2):
            # cb/nb: column bases of the pcur/pnxt regions inside pab
            k, tl = divmod(t, TCH)
            yy = yyp.tile([128, 128], BF16, tag="yy")
            nc.tensor.transpose(yy[:], yy_view(k, tl), idf[:])
            if rec2 is not None:
                # absorb last rescale: pcur *= e^OFFS/max, in place
                nc.vector.tensor_scalar_mul(pab[:, cb + 1:cb + 130],
                                            pab[:, cb + 1:cb + 130], rec2[:])
            # A[s] = R[s] + e^-g R[s-1] -> pnxt (even states final)
            nc.vector.scalar_tensor_tensor(pab[:, nb + 1:nb + 130],
                                           pab[:, cb:cb + 129], E1,
                                           pab[:, cb + 1:cb + 130],
                                           OP.mult, OP.add)
            # x[0,l] = R[2l-1] * yy_skip[l];  x[1,l] = A[2l+1] * yy_lab[l]
            stz = bass.AP(pab[:].tensor, pab[:].offset + cb,
                          [pab[:].ap[0], [nb - cb + 2, 2], [2, 64]])
            x = vpl.tile([128, 128], F32, tag="x")
            nc.vector.tensor_tensor(x[:], stz, yy[:], OP.mult)
            # odd states: overwrite A_odd with yl*A + ys*R[2l-1]
            odd = bass.AP(pab[:].tensor, pab[:].offset + nb + 2,
                          [pab[:].ap[0], [2, 64]])
            nc.vector.tensor_tensor(odd, x[:, 0:64], x[:, 64:128], OP.add)
            if t % RBLK == RBLK - 1 and t < NRES * RBLK:
                ridx = t // RBLK
                mxc = mxh[:, ridx:ridx + 1]
                nc.vector.tensor_reduce(mxc, pab[:, nb + 1:nb + 130],
                                        AX.X, OP.max)
                rec = spl.tile([128, 1], F32, tag="rec")
                nc.vector.reciprocal(rec[:], mxc)
                rec2n = spl.tile([128, 1], F32, tag="rec2")
                nc.vector.tensor_scalar(rec2n[:], rec[:], OFFE, None, OP.mult)
                return rec2n
            return None

        # ---- chunk 0: load + gather before the DP; y DMAs split by
        # b-quarters so the first matmuls start after ~1/4 of the DMA ----
        lo0 = ypl.tile([128, BL * TCH], BF16, tag="ypt", name="ypt0lo")
        hi0 = ypl.tile([128, BL * TCH], BF16, tag="ypt", name="ypt0hi")
        for bq in range(4):
            bs = slice(bq * 32 * TCH, (bq + 1) * 32 * TCH)
            nc.sync.dma_start(lo0[:, bs],
                              ypt_d[0, 0, :, bq * 32:(bq + 1) * 32]
                              .rearrange("c b t -> c (b t)"))
            nc.sync.dma_start(hi0[:, bs],
                              ypt_d[0, 1, :, bq * 32:(bq + 1) * 32]
                              .rearrange("c b t -> c (b t)"))
        yp_t[0] = {0: lo0, 1: hi0}
        gc_t[0] = (g0a, g0b)
        for q in range(NQ):
            if q % 4 == 0:
                wg_load(0, q // 4)
            for si in range(4):
                gather_si(0, q, si, sub=0)
            for si in range(4):
                gather_si(0, q, si, sub=1)

        # init (t = 0): R[s=0] = 1; R~[s=1] = e^-g * yhat_lab(l=0,t=0)
        yy0 = yyp.tile([128, 128], BF16, tag="yy")
        nc.tensor.transpose(yy0[:], yy_view(0, 0), idf[:])
        nc.vector.memset(pab[:, 1:2], 1.0)
        nc.vector.tensor_scalar(pab[:, 2:3], yy0[:, 64:65], E1, None, OP.mult)

        cb, nb = 0, PB
        rec2 = None
        for t in range(1, T):
            k, tl = divmod(t, TCH)
            # interleave gather work through the DP steps so the PE keeps
            # just ahead of the per-step yy transposes
            if k == 0:
                # steps 1..43: chunk-0 second half (3 samples/step);
                # steps 64..127: chunk 1 (2 samples/step). Chunk-1 y tiles
                # stream in as the chunk-0 tiles free up (ypl bufs=3).
                if tl == 44:
                    ypt_load(1)
                    g = gcp.tile([128, TCH * 128], BF16, tag="gc", name="gc")
                    gc_t[1] = g
                if 64 <= tl <= 127:
                    if (tl - 64) % 8 == 0:
                        wg_load(1, (tl - 64) // 8)
                    u = 2 * (tl - 64)
                    gather_si(1, u // 4, u % 4)
                    gather_si(1, (u + 1) // 4, (u + 1) % 4)
            elif k + 1 < NCH:
                if tl == 0:
                    ypt_load(k + 1)
                    g = gcp.tile([128, TCH * 128], BF16, tag="gc", name="gc")
                    gc_t[k + 1] = g
                if tl % 16 == 0:
                    wg_load(k + 1, tl // 16)
                gather_si(k + 1, tl // 4, tl % 4)
            rec2 = dp_step(t, cb, nb, rec2)
            cb, nb = nb, cb
        if rec2 is not None:
            nc.vector.tensor_scalar_mul(pab[:, cb + 1:cb + 130],
                                        pab[:, cb + 1:cb + 130], rec2[:])

        # final: pend = sum(R * endmask); exact logs happen on the host
        scre = per.tile([128, S], F32, tag="scre", name="scre")
        nc.vector.tensor_tensor(scre[:], pab[:, cb + 1:cb + 130], em_sb[:],
                                OP.mult)
        pend = per.tile([128, 1], F32, tag="pend", name="pend")
        nc.vector.tensor_reduce(pend[:], scre[:], AX.X, OP.add)
        nc.sync.dma_start(pend_d, pend[:])
        nc.sync.dma_start(mxh_d, mxh[:])

    nc.compile()
    return nc


def _host_derived(y_true, label_length):
    import ml_dtypes

    lab = np.asarray(y_true, dtype=np.int64)  # [B, 64]
    llv = np.asarray(label_length).reshape(-1)
    # packed one-hots: [B, C, 128]; cols 0..63 skip-masked labels scaled by
    # e^(-2g), cols 64..127 labels (validity-masked)
    vm = (np.arange(L)[None, :] < llv[:, None])  # valid odd state s=2l+1
    zm = np.concatenate([np.zeros((B, 1), bool), lab[:, 1:] != lab[:, :-1]], axis=1)
    w = np.zeros((B, C, 128), dtype=np.float32)
    bb = np.repeat(np.arange(B), L)
    ll = np.tile(np.arange(L), B)
    cc = lab.reshape(-1)
    w[bb, cc, L + ll] = vm.reshape(-1).astype(np.float32)
    w[bb, cc, ll] = np.where(
        (zm & vm).reshape(-1),
        np.float32(np.exp(-2.0 * G_TILT)),
        w[bb, cc, ll],
    )
    # device layout: [group, 128c(lo), 4q, 4si, (ck m)] with c = ck*128 + c_lo
    w6 = w.reshape(B // 16, 4, 4, 2, 128, 128)      # [g, q4, si, ck, c_lo, m]
    w6 = w6.transpose(0, 4, 1, 2, 3, 5)             # [g, c_lo, q4, si, ck, m]
    wgg = np.ascontiguousarray(
        w6.reshape(B // 16, 128, 4 * 4 * 256).astype(ml_dtypes.bfloat16)
    )
    return wgg


def kernel(y_true, y_pred, input_length, label_length, _trace=False):
    global _prog, _last_results
    from concourse.bass_utils import run_bass_kernel_spmd

    y_true = np.asarray(y_true)
    import ml_dtypes
    y_pred = np.asarray(y_pred, dtype=np.float32)
    label_length = np.asarray(label_length).reshape(-1)

    # blank-normalized emissions; the blank product returns via logbeta
    ybe = y_pred[:, :, C - 1] + np.float32(EPS)          # [B, T]
    yhat = (y_pred + np.float32(EPS)) / ybe[:, :, None]
    yhat_bf = yhat.astype(ml_dtypes.bfloat16)
    logbeta = np.log(ybe.astype(np.float64)).sum(axis=1)  # [B]

    TCH = 128
    NCH = T // TCH
    # ypt[core][k,h,c,b,t] = yhat[b, k*TCH+t, h*128+c]
    ypt = yhat_bf.reshape(NCORES, BL, NCH, TCH, 2, 128).transpose(
        0, 2, 4, 5, 1, 3)
    ypt = np.ascontiguousarray(ypt)

    wgg = _host_derived(y_true, label_length)
    em = np.zeros((B, S), dtype=np.float32)
    bidx = np.arange(B)
    em[bidx, 2 * label_length] = 1.0
    em[bidx, 2 * label_length - 1] = np.float32(np.exp(-G_TILT))
    idf = np.eye(128, dtype=ml_dtypes.bfloat16)

    if _prog is None:
        _prog = _build_program()

    NGC = BL // 16  # wg groups per core
    in_maps = []
    for i in range(NCORES):
        sl = slice(i * BL, (i + 1) * BL)
        slg = slice(i * NGC, (i + 1) * NGC)
        in_maps.append({
            "ypt": ypt[i],
            "wgg": wgg[slg],
            "em": em[sl],
            "idf": idf,
        })
    res = run_bass_kernel_spmd(_prog, in_maps, core_ids=list(range(NCORES)),
                               trace=_trace)
    _last_results = res
    pend = np.concatenate([r["pend"] for r in res.results], axis=0).reshape(-1)
    mxh = np.concatenate([r["mxh"] for r in res.results], axis=0)
    logacc = np.log(mxh.astype(np.float64)).sum(axis=1) - OFFS * NRES
    loss = -(np.log(pend.astype(np.float64)) + logacc + logbeta
             + G_TILT * 2.0 * label_length.astype(np.float64))
    return loss.reshape(B, 1).astype(np.float32)


if __name__ == "__main__":
    rng = np.random.default_rng(0)
    yp = rng.random((B, T, C), dtype=np.float32)
    yp /= yp.sum(-1, keepdims=True)
    yt = rng.integers(0, C - 1, size=(B, L)).astype(np.int32)
    il = np.full((B, 1), T, dtype=np.int32)
    ll = rng.integers(32, L + 1, size=(B, 1)).astype(np.int32)
    print(kernel(yt, yp, il, ll)[:4])
